# revision 9
# baseline (speedup 1.0000x reference)
"""Distributed causal MultiHeadAttention kernel for 8 Trainium2 NeuronCores.

Problem: B=4, S=2048, D=1024, H=16 heads, dk=dv=64, causal mask, fp32 I/O.

Sharding: data-parallel over batch (4) x tensor-parallel over heads (2 groups
of 8) = 8 cores. Core c handles batch c//2 with heads (c%2)*8 .. (c%2)*8+7.
Each core computes a partial output [S, D] (its head group's contribution
through the corresponding w_o rows); the host sums the pair of partials per
batch (the "all-reduce" of the output projection, done host-side).

Device dataflow (bf16 matmuls with fp32 PSUM accumulation, except q/k
projections which run fp8e4m3 in DoubleRow perf mode = 4x PE throughput;
w_q/w_k are pre-scaled x64 host-side and the 1/64^2 is folded into the
softmax exp scale, which keeps the fp8 quantization noise confined to the
logits where softmax normalization damps it; measured end-to-end absmax
rel err ~1.1% vs the 2% gate):

  - The whole schedule is organized so the ScalarE exp stream (the real
    bottleneck: ~139k elems/lane at 1.2 GHz ~= 130+ us) never starves:
    only the first 512-row tile of kT/qT/v is projected up front (~10 us
    incl. its input DMA, issued st-major), then attention starts and ALL
    remaining projection work + the per-q-tile output projections are
    emitted as small "fill units" interleaved one-per-k-chunk into the
    attention instruction stream, so the PE executes them inside the gaps
    of the ACT-bound attention phase instead of serializing in front of it.
  - qT/kT = w.T @ xT -> [512, S] head-major rows (fp8 DoubleRow chains).
  - v = xT.T @ wv -> [S, 512] with a constant 1.0 column per head
    ([S, 8, 65]) so A@V also produces softmax row sums ("ones trick").
  - Scores per head pair, transposed: S^T[k, q] = kT.T @ qT; the even/odd
    head rows sit at partitions 0-63 / 64-127 so the two dk=64 matmuls
    row-tile onto disjoint PE quadrants and run concurrently.
  - One exp per (pair, q-tile, k-chunk) on ScalarE straight out of PSUM
    (scale folded in; no max subtraction - scores are O(1) bounded).
    Causal mask: fully-masked column ranges are skipped outright, the
    triangular 128x128 diagonal block is zeroed post-exp via gpsimd
    affine_select.
  - out^T[dv(+1), q] accumulated over k-chunks: lhsT = [V_h | 1], rhs = A^T.
    Row 64 is the softmax denominator r[q]; per head pair the reciprocal is
    taken on DVE and broadcast across partitions with an SBUF->SBUF DMA.
  - Output projection per q-tile (oT.T @ wo) is queued as fill units right
    after that q-tile's normalization.
"""

import collections

import numpy as np
import ml_dtypes

import concourse.bass as bass
import concourse.bacc as bacc
import concourse.mybir as mybir
import concourse.tile as tile
from concourse.bass_utils import run_bass_kernel_spmd

B, S, D = 4, 2048, 1024
H, DK = 16, 64
HL = 8              # heads handled per core
NHL = HL * DK       # 512 rows of head-dim per core
P = 128
NCORES = 8
ST = 512            # q-tile width (matmul free dim / PSUM bank)
NQT = S // ST       # 4
NKC = S // P        # 16 k chunks
MC = NHL // P       # 4 head-pair chunks
DC = D // P         # 8 chunks of D
DCP = DC // 2       # 4 DoubleRow pair-chunks of D

FP32 = mybir.dt.float32
BF16 = mybir.dt.bfloat16
FP8 = mybir.dt.float8e4
EXP = mybir.ActivationFunctionType.Exp
DR = mybir.MatmulPerfMode.DoubleRow

WS = 64.0                        # host-side pre-scale on w_q / w_k
EXP_SCALE = 0.125 / (WS * WS)    # 1/sqrt(dk) / ws^2 = 2^-15, exact


def _emit(tc):
    nc = tc.nc

    # x inputs are st-major so each 512-column slice is one contiguous
    # 4-8 KB-per-partition DMA (512 B descriptors run ~8x slower)
    xq8 = nc.dram_tensor("xq8", [NQT, P, DCP, 2, ST], FP8, kind="ExternalInput").ap()
    xk8 = nc.dram_tensor("xk8", [NQT, P, DCP, 2, ST], FP8, kind="ExternalInput").ap()
    xvT = nc.dram_tensor("xvT", [NQT, P, DC, ST], BF16, kind="ExternalInput").ap()
    wq8 = nc.dram_tensor("wq8", [P, DCP, 2, NHL], FP8, kind="ExternalInput").ap()
    wk8 = nc.dram_tensor("wk8", [P, DCP, 2, NHL], FP8, kind="ExternalInput").ap()
    wv = nc.dram_tensor("wv", [P, DC, NHL], BF16, kind="ExternalInput").ap()
    wo = nc.dram_tensor("wo", [P, MC, D], BF16, kind="ExternalInput").ap()
    out = nc.dram_tensor("out", [S, D], FP32, kind="ExternalOutput").ap()

    with (
        tc.tile_pool(name="sing", bufs=1) as sing,
        tc.tile_pool(name="apool", bufs=6) as apool,
        tc.tile_pool(name="rpool", bufs=2) as rpool,
        tc.tile_pool(name="outp", bufs=3) as outp,
        tc.tile_pool(name="dpool", bufs=3, space="DRAM") as dpool,
        tc.tile_pool(name="psS", bufs=2, space="PSUM") as psS,
        tc.tile_pool(name="psO", bufs=2, space="PSUM") as psO,
        tc.tile_pool(name="psP", bufs=2, space="PSUM") as psP,
    ):
        # ---- persistent SBUF tiles -------------------------------------
        wq_sb = sing.tile([P, DCP, 2, NHL], FP8, tag="wq_sb")
        wk_sb = sing.tile([P, DCP, 2, NHL], FP8, tag="wk_sb")
        wv_sb = sing.tile([P, DC, NHL], BF16, tag="wv_sb")
        wo_sb = sing.tile([P, MC, D], BF16, tag="wo_sb")
        xq_sb = sing.tile([P, NQT, DCP, 2, ST], FP8, tag="xq_sb")
        xk_sb = sing.tile([P, NQT, DCP, 2, ST], FP8, tag="xk_sb")
        xv_sb = sing.tile([P, NQT, DC, ST], BF16, tag="xv_sb")
        qT = sing.tile([P, MC, S], BF16, tag="qT")
        kT = sing.tile([P, MC, S], BF16, tag="kT")
        v65 = sing.tile([P, NKC, HL, DK + 1], BF16, tag="v65")
        oT = sing.tile([P, MC, S], BF16, tag="oT")

        nc.gpsimd.memset(v65[:, :, :, DK : DK + 1], 1.0)

        # ---- DMA issue order = first-need order (st-major for x).
        # Two queues: the q/k path on the SP ring, the v path + wo on the
        # Pool ring, so xv st0 lands in time for the JIT v-projection fill
        # units instead of queueing behind 2 MB of q/k input.
        # The st0 w/x transfers are split per-dcp so the first kT matmul
        # only waits on its own 256 KB instead of the full 2 MB preamble.
        for dcp in range(DCP):
            nc.sync.dma_start(wk_sb[:, dcp], wk8[:, dcp])
            nc.sync.dma_start(xk_sb[:, 0, dcp], xk8[0, :, dcp])
        for dcp in range(DCP):
            nc.sync.dma_start(wq_sb[:, dcp], wq8[:, dcp])
            nc.sync.dma_start(xq_sb[:, 0, dcp], xq8[0, :, dcp])
        nc.gpsimd.dma_start(wv_sb, wv)
        for st in range(NQT):
            nc.gpsimd.dma_start(xv_sb[:, st], xvT[st])
        nc.gpsimd.dma_start(wo_sb, wo)
        for st in range(1, NQT):
            nc.sync.dma_start(xk_sb[:, st], xk8[st])
            nc.sync.dma_start(xq_sb[:, st], xq8[st])

        # ---- projection units ------------------------------------------
        def proj_qk(w_sb, x_sb, dst, st, mc):
            ps = psP.tile([P, ST], FP32, tag="psP")
            for dcp in range(DCP):
                nc.tensor.matmul(
                    ps,
                    lhsT=w_sb[:, dcp, :, mc * P : (mc + 1) * P],
                    rhs=x_sb[:, st, dcp, :, :],
                    start=(dcp == 0),
                    stop=(dcp == DCP - 1),
                    perf_mode=DR,
                )
            nc.vector.tensor_copy(dst[:, mc, st * ST : (st + 1) * ST], ps)

        def proj_v(sc):
            st, j = divmod(sc, ST // P)
            ps = psP.tile([P, ST], FP32, tag="psP")
            for dc in range(DC):
                nc.tensor.matmul(
                    ps,
                    lhsT=xv_sb[:, st, dc, j * P : (j + 1) * P],
                    rhs=wv_sb[:, dc, :],
                    start=(dc == 0),
                    stop=(dc == DC - 1),
                )
            nc.vector.tensor_copy(
                v65[:, sc, :, 0:DK], ps.rearrange("p (h d) -> p h d", h=HL)
            )

        def o_proj(sc, nt):
            ps = psP.tile([P, ST], FP32, tag="psP")
            for c in range(MC):
                nc.tensor.matmul(
                    ps,
                    lhsT=oT[:, c, sc * P : (sc + 1) * P],
                    rhs=wo_sb[:, c, nt * ST : (nt + 1) * ST],
                    start=(c == 0),
                    stop=(c == MC - 1),
                )
            ob = outp.tile([P, ST], FP32, tag="ob")
            nc.vector.tensor_copy(ob, ps)
            nc.sync.dma_start(
                out[sc * P : (sc + 1) * P, nt * ST : (nt + 1) * ST], ob
            )

        # preamble: only what q-tile 0's first scores need
        for mc in range(MC):
            proj_qk(wk_sb, xk_sb, kT, 0, mc)
        for mc in range(MC):
            proj_qk(wq_sb, xq_sb, qT, 0, mc)

        # deferred PE work, drip-fed into the ACT-bound attention stream.
        # v chunks 0-3 first: A@V for (qt0, kc) needs v chunk kc, and the
        # first A@V is emitted one step after the first exp. qT st1 before
        # kT st1: q-tile 1's first scores read qT st1, while kT st1 isn't
        # touched until its 5th k-chunk.
        # CAUTION: a unit must be POPPED (emitted) before the first attention
        # instruction that reads its output is emitted — emission order is
        # what the dependency tracker sees; a consumer emitted first reads
        # stale data with no ordering at all.
        fill = collections.deque()
        for sc in range(ST // P):
            fill.append((lambda sc=sc: proj_v(sc)))
        for sc in range(ST // P, 2 * (ST // P)):
            fill.append((lambda sc=sc: proj_v(sc)))
        for mc in range(MC):
            fill.append((lambda mc=mc: proj_qk(wq_sb, xq_sb, qT, 1, mc)))
        for mc in range(MC):
            fill.append((lambda mc=mc: proj_qk(wk_sb, xk_sb, kT, 1, mc)))
        for st in range(2, NQT):
            for mc in range(MC):
                fill.append((lambda st=st, mc=mc: proj_qk(wq_sb, xq_sb, qT, st, mc)))
            for mc in range(MC):
                fill.append((lambda st=st, mc=mc: proj_qk(wk_sb, xk_sb, kT, st, mc)))
            for sc in range(st * (ST // P), (st + 1) * (ST // P)):
                fill.append((lambda sc=sc: proj_v(sc)))

        tick = [0]

        def pump(n=1):
            for _ in range(n):
                if fill:
                    fill.popleft()()

        pump(2)  # v chunks 0-1 ahead of the first A@V

        # ---- attention + interleaved fill --------------------------------
        def clo(kc, qt):
            j = kc - qt * (ST // P)
            return j * P if j > 0 else 0

        def emit_av(a_t, kc, oT_ps, pc, qt, nkc):
            lo = clo(kc, qt)
            for hh in range(2):
                nc.tensor.matmul(
                    oT_ps[hh][:, lo:ST],
                    lhsT=v65[:, kc, 2 * pc + hh, :],
                    rhs=a_t[:, hh, lo:ST],
                    start=(kc == 0),
                    stop=(kc == nkc - 1),
                )

        pending = []      # previous q-tile's o_proj units
        reserve = []      # units held back to cover the final norm window
        for qt in range(NQT):
            for pc in range(MC):
                # release the previous q-tile's output projection only one
                # bucket in, after its normalization has landed, so those
                # units never head-of-line-block the PE queue. For the last
                # q-tile, half the units are held in reserve to keep the PE
                # fed while the final pc's normalization chain completes.
                if pc == 1 and pending:
                    if qt == NQT - 1:
                        fill.extend(pending[: len(pending) // 2])
                        reserve = pending[len(pending) // 2 :]
                    else:
                        fill.extend(pending)
                    pending = []
                nkc = (qt + 1) * (ST // P)
                oT_ps = [
                    psO.tile([DK + 1, ST], FP32, tag="psO", name=f"psO_{hh}")
                    for hh in range(2)
                ]
                prev = []
                for kc in range(nkc):
                    lo = clo(kc, qt)
                    sps = psS.tile([P, 2, ST], FP32, tag="psS", name="sps")
                    for hh in range(2):
                        pp = hh * 64
                        nc.tensor.matmul(
                            sps[:, hh, lo:ST],
                            lhsT=kT[pp : pp + 64, pc, kc * P : (kc + 1) * P],
                            rhs=qT[pp : pp + 64, pc, qt * ST + lo : (qt + 1) * ST],
                            start=True,
                            stop=True,
                        )
                    a_t = apool.tile([P, 2, ST], BF16, tag="a", name="a_t")
                    nc.scalar.activation(
                        a_t[:, :, lo:ST], sps[:, :, lo:ST], EXP,
                        bias=0.0, scale=EXP_SCALE,
                    )
                    if kc >= qt * (ST // P):
                        # triangular block: keep where q_local >= k_local
                        nc.gpsimd.affine_select(
                            out=a_t[:, :, lo : lo + P],
                            in_=a_t[:, :, lo : lo + P],
                            pattern=[[0, 2], [1, P]],
                            channel_multiplier=-1,
                            base=0,
                            compare_op=mybir.AluOpType.is_ge,
                            fill=0.0,
                        )
                    prev.append((a_t, kc))
                    if len(prev) > 1:
                        a_p, kc_p = prev.pop(0)
                        emit_av(a_p, kc_p, oT_ps, pc, qt, nkc)
                    # pacing: q-tile 0 pumps every step so all 16 st0/st1
                    # units are emitted before q-tile 1's consumers; later
                    # q-tiles pump one unit per two k-chunks, matching the
                    # per-step ACT slack so the exp stream stays packed
                    tick[0] += 1
                    if qt == 0 or tick[0] % 2 == 0:
                        pump(1)
                for a_p, kc_p in prev:
                    emit_av(a_p, kc_p, oT_ps, pc, qt, nkc)

                # per-pc normalization: copy the raw (unnormalized) oT out,
                # take the reciprocal of the two softmax-sum rows directly on
                # DVE, broadcast across partitions with an SBUF->SBUF DMA and
                # normalize on the Pool engine. The whole chain is off the PE
                # path and hides behind the next pc's attention, so no q-tile
                # boundary ever stalls the PE (which would re-throttle HAM).
                rd = dpool.tile([2, ST], FP32, tag="rd", name="rd")
                for hh in range(2):
                    ps = oT_ps[hh]
                    rsb = rpool.tile([1, ST], FP32, tag=f"rsb{hh}")
                    nc.vector.tensor_copy(rsb, ps[DK : DK + 1, :])
                    nc.vector.tensor_copy(
                        oT[hh * 64 : (hh + 1) * 64, pc, qt * ST : (qt + 1) * ST],
                        ps[0:DK, :],
                    )
                    ri = rpool.tile([1, ST], FP32, tag=f"ri{hh}")
                    nc.vector.reciprocal(ri, rsb)
                    # partition-broadcast needs a DRAM source (zero partition
                    # step is illegal for SBUF APs), so bounce ri through
                    # DRAM; all hops stay on the sync ring for ordering.
                    nc.sync.dma_start(rd[hh : hh + 1, :], ri)
                rrep = rpool.tile([P, ST], FP32, tag="rrep")
                for hh in range(2):
                    nc.sync.dma_start(
                        rrep[hh * 64 : (hh + 1) * 64, :],
                        rd[hh : hh + 1, :].to_broadcast((64, ST)),
                    )
                sl = oT[:, pc, qt * ST : (qt + 1) * ST]
                nc.gpsimd.tensor_mul(sl, sl, rrep)

            # hold this q-tile's output projection back until one bucket
            # into the next q-tile (see `pending` release above)
            pending = [
                (lambda sc=sc, nt=nt: o_proj(sc, nt))
                for sc in range(qt * (ST // P), (qt + 1) * (ST // P))
                for nt in range(D // ST)
            ]

        # tail: the reserved units (already normalized long ago) keep the PE
        # busy while the last pc's normalization chain completes, then the
        # final q-tile's own output projections drain.
        fill.extend(reserve)
        fill.extend(pending)
        pump(len(fill))


_CACHE = {}


def build_nc():
    if "nc" not in _CACHE:
        # Bacc (not plain Bass): its finalize runs the pass pipeline that
        # splits multi-semaphore waits into event-semaphore/ldweights slots,
        # which walrus requires (max 1 wait per instruction on TRN2).
        nc = bacc.Bacc()
        with tile.TileContext(nc) as tc:
            _emit(tc)
        nc.finalize()
        _CACHE["nc"] = nc
    return _CACHE["nc"]


def make_in_maps(query, key, value, w_q, w_k, w_v, w_o):
    bf = ml_dtypes.bfloat16
    e4 = ml_dtypes.float8_e4m3

    def packT(x):  # [S, D] fp32 -> xT packed [NQT, 128, DC, ST] bf16 st-major
        xb = np.asarray(x, np.float32).astype(bf)
        return np.ascontiguousarray(
            xb.T.reshape(DC, P, NQT, ST).transpose(2, 1, 0, 3)
        )

    def packX8(x):  # [S, D] fp32 -> [NQT, 128, DCP, 2, ST] e4m3 st-major
        xb = np.asarray(x, np.float32).T.astype(e4)  # [D, S]
        return np.ascontiguousarray(
            xb.reshape(DCP, 2, P, NQT, ST).transpose(3, 2, 0, 1, 4)
        )

    def packW8(w):  # [D, NHL] -> [128, DCP, 2, NHL] e4m3, pre-scaled x64
        wb = (np.asarray(w, np.float32) * WS).astype(e4)
        return np.ascontiguousarray(wb.reshape(DCP, 2, P, NHL).transpose(2, 0, 1, 3))

    def packW(w):  # [D, NHL] -> [128, DC, NHL]
        wb = np.asarray(w, np.float32).astype(bf)
        return np.ascontiguousarray(wb.reshape(DC, P, NHL).transpose(1, 0, 2))

    def packWo(w):  # [NHL, D] -> [128, MC, D]
        wb = np.asarray(w, np.float32).astype(bf)
        return np.ascontiguousarray(wb.reshape(MC, P, D).transpose(1, 0, 2))

    query = np.asarray(query, np.float32)
    key = np.asarray(key, np.float32)
    value = np.asarray(value, np.float32)
    in_maps = []
    for c in range(NCORES):
        b, hg = divmod(c, 2)
        cols = slice(hg * NHL, (hg + 1) * NHL)
        in_maps.append(
            {
                "xq8": packX8(query[b]),
                "xk8": packX8(key[b]),
                "xvT": packT(value[b]),
                "wq8": packW8(np.asarray(w_q)[:, cols]),
                "wk8": packW8(np.asarray(w_k)[:, cols]),
                "wv": packW(np.asarray(w_v)[:, cols]),
                "wo": packWo(np.asarray(w_o)[cols, :]),
            }
        )
    return in_maps


def kernel(query, key, value, mask, w_q, w_k, w_v, w_o, **run_kwargs):
    nc = build_nc()
    in_maps = make_in_maps(query, key, value, w_q, w_k, w_v, w_o)
    res = run_bass_kernel_spmd(nc, in_maps, list(range(NCORES)), **run_kwargs)
    out = np.empty((B, S, D), np.float32)
    for b in range(B):
        out[b] = res.results[2 * b]["out"] + res.results[2 * b + 1]["out"]
    return out



# revision 15
# speedup vs baseline: 1.3148x; 1.3148x over previous
"""Distributed causal MultiHeadAttention kernel for 8 Trainium2 NeuronCores.

Problem: B=4, S=2048, D=1024, H=16 heads, dk=dv=64, causal mask, fp32 I/O.

Sharding: data-parallel over batch (4) x tensor-parallel over heads (2 groups
of 8) = 8 cores. Core c handles batch c//2 with heads (c%2)*8 .. (c%2)*8+7.
Each core computes a partial output [S, D] (its head group's contribution
through the corresponding w_o rows); the host sums the pair of partials per
batch (the "all-reduce" of the output projection, done host-side).

Device dataflow (bf16 matmuls with fp32 PSUM accumulation, except q/k
projections which run fp8e4m3 in DoubleRow perf mode = 4x PE throughput;
w_q/w_k are pre-scaled x64 host-side and the 1/64^2 is folded into the
softmax exp scale, which keeps the fp8 quantization noise confined to the
logits where softmax normalization damps it; measured end-to-end absmax
rel err ~1.1% vs the 2% gate):

  - The whole schedule is organized so the ScalarE exp stream (the real
    bottleneck: ~139k elems/lane at 1.2 GHz ~= 130+ us) never starves:
    only the first 512-row tile of kT/qT/v is projected up front (~10 us
    incl. its input DMA, issued st-major), then attention starts and ALL
    remaining projection work + the per-q-tile output projections are
    emitted as small "fill units" interleaved one-per-k-chunk into the
    attention instruction stream, so the PE executes them inside the gaps
    of the ACT-bound attention phase instead of serializing in front of it.
  - qT/kT = w.T @ xT -> [512, S] head-major rows (fp8 DoubleRow chains).
  - v = xT.T @ wv -> [S, 512] with a constant 1.0 column per head
    ([S, 8, 65]) so A@V also produces softmax row sums ("ones trick").
  - Scores per head pair, transposed: S^T[k, q] = kT.T @ qT; the even/odd
    head rows sit at partitions 0-63 / 64-127 so the two dk=64 matmuls
    row-tile onto disjoint PE quadrants and run concurrently.
  - One exp per (pair, q-tile, k-chunk) on ScalarE straight out of PSUM
    (scale folded in; no max subtraction - scores are O(1) bounded).
    Causal mask: fully-masked column ranges are skipped outright, the
    triangular 128x128 diagonal block is zeroed post-exp via gpsimd
    affine_select.
  - out^T[dv(+1), q] accumulated over k-chunks: lhsT = [V_h | 1], rhs = A^T.
    Row 64 is the softmax denominator r[q]; per head pair the reciprocal is
    taken on DVE and broadcast across partitions with an SBUF->SBUF DMA.
  - Output projection per q-tile (oT.T @ wo) is queued as fill units right
    after that q-tile's normalization.
"""

import collections

import numpy as np
import ml_dtypes

import concourse.bass as bass
import concourse.bacc as bacc
import concourse.mybir as mybir
import concourse.tile as tile
from concourse.bass_utils import run_bass_kernel_spmd

B, S, D = 4, 2048, 1024
H, DK = 16, 64
HL = 8              # heads handled per core
NHL = HL * DK       # 512 rows of head-dim per core
P = 128
NCORES = 8
ST = 512            # q-tile width (matmul free dim / PSUM bank)
NQT = S // ST       # 4
NKC = S // P        # 16 k chunks
MC = NHL // P       # 4 head-pair chunks
DC = D // P         # 8 chunks of D
DCP = DC // 2       # 4 DoubleRow pair-chunks of D

FP32 = mybir.dt.float32
BF16 = mybir.dt.bfloat16
FP8 = mybir.dt.float8e4
EXP = mybir.ActivationFunctionType.Exp
DR = mybir.MatmulPerfMode.DoubleRow

WS = 64.0                        # host-side pre-scale on w_q / w_k
EXP_SCALE = 0.125 / (WS * WS)    # 1/sqrt(dk) / ws^2 = 2^-15, exact


def _emit(tc):
    nc = tc.nc

    # x inputs are st-major so each 512-column slice is one contiguous
    # 4-8 KB-per-partition DMA (512 B descriptors run ~8x slower)
    xq8 = nc.dram_tensor("xq8", [NQT, P, DCP, 2, ST], FP8, kind="ExternalInput").ap()
    xk8 = nc.dram_tensor("xk8", [NQT, P, DCP, 2, ST], FP8, kind="ExternalInput").ap()
    xvT = nc.dram_tensor("xvT", [NQT, P, DC, ST], BF16, kind="ExternalInput").ap()
    wq8 = nc.dram_tensor("wq8", [P, DCP, 2, NHL], FP8, kind="ExternalInput").ap()
    wk8 = nc.dram_tensor("wk8", [P, DCP, 2, NHL], FP8, kind="ExternalInput").ap()
    wv = nc.dram_tensor("wv", [P, DC, NHL], BF16, kind="ExternalInput").ap()
    wo = nc.dram_tensor("wo", [P, MC, D], BF16, kind="ExternalInput").ap()
    out = nc.dram_tensor("out", [S, D], FP32, kind="ExternalOutput").ap()

    with (
        tc.tile_pool(name="sing", bufs=1) as sing,
        tc.tile_pool(name="apool", bufs=6) as apool,
        tc.tile_pool(name="rpool", bufs=2) as rpool,
        tc.tile_pool(name="outp", bufs=3) as outp,
        tc.tile_pool(name="dpool", bufs=3, space="DRAM") as dpool,
        tc.tile_pool(name="psS", bufs=2, space="PSUM") as psS,
        tc.tile_pool(name="psO", bufs=2, space="PSUM") as psO,
        tc.tile_pool(name="psP", bufs=2, space="PSUM") as psP,
    ):
        # ---- persistent SBUF tiles -------------------------------------
        wq_sb = sing.tile([P, DCP, 2, NHL], FP8, tag="wq_sb")
        wk_sb = sing.tile([P, DCP, 2, NHL], FP8, tag="wk_sb")
        wv_sb = sing.tile([P, DC, NHL], BF16, tag="wv_sb")
        wo_sb = sing.tile([P, MC, D], BF16, tag="wo_sb")
        xq_sb = sing.tile([P, NQT, DCP, 2, ST], FP8, tag="xq_sb")
        xk_sb = sing.tile([P, NQT, DCP, 2, ST], FP8, tag="xk_sb")
        xv_sb = sing.tile([P, NQT, DC, ST], BF16, tag="xv_sb")
        qT = sing.tile([P, MC, S], BF16, tag="qT")
        kT = sing.tile([P, MC, S], BF16, tag="kT")
        v65 = sing.tile([P, NKC, HL, DK + 1], BF16, tag="v65")
        oT = sing.tile([P, MC, S], BF16, tag="oT")

        nc.gpsimd.memset(v65[:, :, :, DK : DK + 1], 1.0)
        warm = sing.tile([P, P], BF16, tag="warm")
        nc.gpsimd.memset(warm, 0.0)

        # ---- DMA issue order = first-need order (st-major for x).
        # Two queues: the q/k path on the SP ring, the v path + wo on the
        # Pool ring, so xv st0 lands in time for the JIT v-projection fill
        # units instead of queueing behind 2 MB of q/k input.
        nc.sync.dma_start(wk_sb, wk8)
        nc.sync.dma_start(xk_sb[:, 0], xk8[0])
        nc.sync.dma_start(wq_sb, wq8)
        nc.sync.dma_start(xq_sb[:, 0], xq8[0])
        nc.gpsimd.dma_start(wv_sb, wv)
        for st in range(NQT):
            nc.gpsimd.dma_start(xv_sb[:, st], xvT[st])
        nc.gpsimd.dma_start(wo_sb, wo)
        for st in range(1, NQT):
            nc.sync.dma_start(xk_sb[:, st], xk8[st])
            nc.sync.dma_start(xq_sb[:, st], xq8[st])

        # ~6us of dummy matmuls while the first input DMAs land: trips the
        # PE HAM activity window to K=8/8 so the real stream starts at
        # 2.4 GHz instead of paying the cold 1.2 GHz ramp.
        wps = psP.tile([P, P], FP32, tag="psP")
        for _ in range(64):
            nc.tensor.matmul(wps[0:P, 0:64], lhsT=warm[:, 0:P], rhs=warm[:, 0:64],
                             start=True, stop=True)

        # ---- projection units ------------------------------------------
        def proj_qk(w_sb, x_sb, dst, st, mc):
            ps = psP.tile([P, ST], FP32, tag="psP")
            for dcp in range(DCP):
                nc.tensor.matmul(
                    ps,
                    lhsT=w_sb[:, dcp, :, mc * P : (mc + 1) * P],
                    rhs=x_sb[:, st, dcp, :, :],
                    start=(dcp == 0),
                    stop=(dcp == DCP - 1),
                    perf_mode=DR,
                )
            nc.vector.tensor_copy(dst[:, mc, st * ST : (st + 1) * ST], ps)

        def proj_v(sc):
            st, j = divmod(sc, ST // P)
            ps = psP.tile([P, ST], FP32, tag="psP")
            for dc in range(DC):
                nc.tensor.matmul(
                    ps,
                    lhsT=xv_sb[:, st, dc, j * P : (j + 1) * P],
                    rhs=wv_sb[:, dc, :],
                    start=(dc == 0),
                    stop=(dc == DC - 1),
                )
            nc.vector.tensor_copy(
                v65[:, sc, :, 0:DK], ps.rearrange("p (h d) -> p h d", h=HL)
            )

        def o_proj(sc, nt):
            ps = psP.tile([P, ST], FP32, tag="psP")
            for c in range(MC):
                nc.tensor.matmul(
                    ps,
                    lhsT=oT[:, c, sc * P : (sc + 1) * P],
                    rhs=wo_sb[:, c, nt * ST : (nt + 1) * ST],
                    start=(c == 0),
                    stop=(c == MC - 1),
                )
            ob = outp.tile([P, ST], FP32, tag="ob")
            nc.vector.tensor_copy(ob, ps)
            nc.sync.dma_start(
                out[sc * P : (sc + 1) * P, nt * ST : (nt + 1) * ST], ob
            )

        # preamble: only what q-tile 0's first scores need
        for mc in range(MC):
            proj_qk(wk_sb, xk_sb, kT, 0, mc)
        for mc in range(MC):
            proj_qk(wq_sb, xq_sb, qT, 0, mc)

        # deferred PE work, drip-fed into the ACT-bound attention stream.
        # v chunks 0-3 first: A@V for (qt0, kc) needs v chunk kc, and the
        # first A@V is emitted one step after the first exp. qT st1 before
        # kT st1: q-tile 1's first scores read qT st1, while kT st1 isn't
        # touched until its 5th k-chunk.
        # CAUTION: a unit must be POPPED (emitted) before the first attention
        # instruction that reads its output is emitted — emission order is
        # what the dependency tracker sees; a consumer emitted first reads
        # stale data with no ordering at all.
        fill = collections.deque()
        for sc in range(ST // P):
            fill.append((lambda sc=sc: proj_v(sc)))
        for sc in range(ST // P, 2 * (ST // P)):
            fill.append((lambda sc=sc: proj_v(sc)))
        for mc in range(MC):
            fill.append((lambda mc=mc: proj_qk(wq_sb, xq_sb, qT, 1, mc)))
        for mc in range(MC):
            fill.append((lambda mc=mc: proj_qk(wk_sb, xk_sb, kT, 1, mc)))
        for st in range(2, NQT):
            for mc in range(MC):
                fill.append((lambda st=st, mc=mc: proj_qk(wq_sb, xq_sb, qT, st, mc)))
            for mc in range(MC):
                fill.append((lambda st=st, mc=mc: proj_qk(wk_sb, xk_sb, kT, st, mc)))
            for sc in range(st * (ST // P), (st + 1) * (ST // P)):
                fill.append((lambda sc=sc: proj_v(sc)))

        tick = [0]

        def pump(n=1):
            for _ in range(n):
                if fill:
                    fill.popleft()()

        pump(2)  # v chunks 0-1 ahead of the first A@V

        # ---- attention + interleaved fill --------------------------------
        def clo(kc, qt):
            j = kc - qt * (ST // P)
            return j * P if j > 0 else 0

        def emit_av(a_t, kc, oT_ps, pc, qt, nkc):
            lo = clo(kc, qt)
            for hh in range(2):
                nc.tensor.matmul(
                    oT_ps[hh][:, lo:ST],
                    lhsT=v65[:, kc, 2 * pc + hh, :],
                    rhs=a_t[:, hh, lo:ST],
                    start=(kc == 0),
                    stop=(kc == nkc - 1),
                )

        pending = []      # previous q-tile's o_proj units
        reserve = []      # units held back to cover the final norm window
        norm_mul = []     # deferred normalizing multiply (see below)
        for qt in range(NQT):
            for pc in range(MC):
                # release the previous q-tile's output projection only one
                # bucket in, after its normalization has landed, so those
                # units never head-of-line-block the PE queue. For the last
                # q-tile, half the units are held in reserve to keep the PE
                # fed while the final pc's normalization chain completes.
                if pc == 1 and pending:
                    if qt == NQT - 1:
                        fill.extend(pending[: len(pending) // 2])
                        reserve = pending[len(pending) // 2 :]
                    else:
                        fill.extend(pending)
                    pending = []
                nkc = (qt + 1) * (ST // P)
                oT_ps = [
                    psO.tile([DK + 1, ST], FP32, tag="psO", name=f"psO_{hh}")
                    for hh in range(2)
                ]
                prev = []
                for kc in range(nkc):
                    lo = clo(kc, qt)
                    sps = psS.tile([P, 2, ST], FP32, tag="psS", name="sps")
                    for hh in range(2):
                        pp = hh * 64
                        nc.tensor.matmul(
                            sps[:, hh, lo:ST],
                            lhsT=kT[pp : pp + 64, pc, kc * P : (kc + 1) * P],
                            rhs=qT[pp : pp + 64, pc, qt * ST + lo : (qt + 1) * ST],
                            start=True,
                            stop=True,
                        )
                    a_t = apool.tile([P, 2, ST], BF16, tag="a", name="a_t")
                    nc.scalar.activation(
                        a_t[:, :, lo:ST], sps[:, :, lo:ST], EXP,
                        bias=0.0, scale=EXP_SCALE,
                    )
                    if kc >= qt * (ST // P):
                        # triangular block: keep where q_local >= k_local
                        nc.gpsimd.affine_select(
                            out=a_t[:, :, lo : lo + P],
                            in_=a_t[:, :, lo : lo + P],
                            pattern=[[0, 2], [1, P]],
                            channel_multiplier=-1,
                            base=0,
                            compare_op=mybir.AluOpType.is_ge,
                            fill=0.0,
                        )
                    prev.append((a_t, kc))
                    if len(prev) > 1:
                        a_p, kc_p = prev.pop(0)
                        emit_av(a_p, kc_p, oT_ps, pc, qt, nkc)
                    # pacing: q-tile 0 pumps every step so all 16 st0/st1
                    # units are emitted before q-tile 1's consumers; later
                    # q-tiles pump one unit per two k-chunks, matching the
                    # per-step ACT slack so the exp stream stays packed
                    tick[0] += 1
                    if qt == 0 or tick[0] % 2 == 0:
                        pump(1)
                for a_p, kc_p in prev:
                    emit_av(a_p, kc_p, oT_ps, pc, qt, nkc)

                # the previous pc's normalizing multiply is emitted only now,
                # after this pc's diagonal-mask affine_selects: the Pool
                # queue is strict FIFO, and the multiply waits on its rrep
                # broadcast DMA, so emitting it earlier would head-of-line-
                # block those affine_selects (which gate A@V, i.e. the PE).
                # By now its broadcast has had a whole kc loop to land.
                if norm_mul:
                    norm_mul.pop(0)()

                # per-pc normalization: copy the raw (unnormalized) oT out,
                # reciprocal the two softmax-sum rows lane-parallel ([128, 8]
                # via a DRAM reshape - a flat [1, 512] DVE reciprocal costs
                # 3.3us), broadcast across partitions from DRAM. The chain is
                # off the PE path and hides behind the next pc's attention,
                # so no q-tile boundary ever stalls the PE.
                rd_q = dpool.tile([2, ST], FP32, tag="rdq", name="rd_q")
                for hh in range(2):
                    ps = oT_ps[hh]
                    rsb = rpool.tile([1, ST], FP32, tag=f"rsb{hh}")
                    nc.vector.tensor_copy(rsb, ps[DK : DK + 1, :])
                    nc.vector.tensor_copy(
                        oT[hh * 64 : (hh + 1) * 64, pc, qt * ST : (qt + 1) * ST],
                        ps[0:DK, :],
                    )
                    nc.sync.dma_start(rd_q[hh : hh + 1, :], rsb)
                r128 = rpool.tile([P, 2 * ST // P], FP32, tag="r128")
                nc.sync.dma_start(r128, rd_q.rearrange("a (p f) -> (a p) f", p=64))
                ri128 = rpool.tile([P, 2 * ST // P], FP32, tag="ri128")
                nc.vector.reciprocal(ri128, r128)
                rd_i = dpool.tile([2, ST], FP32, tag="rdi", name="rd_i")
                nc.sync.dma_start(rd_i.rearrange("a (p f) -> (a p) f", p=64), ri128)
                rrep = rpool.tile([P, ST], FP32, tag="rrep")
                for hh in range(2):
                    nc.sync.dma_start(
                        rrep[hh * 64 : (hh + 1) * 64, :],
                        rd_i[hh : hh + 1, :].to_broadcast((64, ST)),
                    )
                sl = oT[:, pc, qt * ST : (qt + 1) * ST]
                norm_mul.append(
                    lambda sl=sl, rrep=rrep: nc.gpsimd.tensor_mul(sl, sl, rrep)
                )

            # hold this q-tile's output projection back until one bucket
            # into the next q-tile (see `pending` release above)
            pending = [
                (lambda sc=sc, nt=nt: o_proj(sc, nt))
                for sc in range(qt * (ST // P), (qt + 1) * (ST // P))
                for nt in range(D // ST)
            ]

        # tail: emit the last pc's multiply (still waiting on its broadcast),
        # then the reserved units (normalized long ago) keep the PE busy
        # while that chain completes, then the final q-tile's own output
        # projections drain.
        if norm_mul:
            norm_mul.pop(0)()
        fill.extend(reserve)
        fill.extend(pending)
        pump(len(fill))


_CACHE = {}


def build_nc():
    if "nc" not in _CACHE:
        # Bacc (not plain Bass): its finalize runs the pass pipeline that
        # splits multi-semaphore waits into event-semaphore/ldweights slots,
        # which walrus requires (max 1 wait per instruction on TRN2).
        nc = bacc.Bacc()
        with tile.TileContext(nc) as tc:
            _emit(tc)
        nc.finalize()
        _CACHE["nc"] = nc
    return _CACHE["nc"]


def make_in_maps(query, key, value, w_q, w_k, w_v, w_o):
    bf = ml_dtypes.bfloat16
    e4 = ml_dtypes.float8_e4m3

    def packT(x):  # [S, D] fp32 -> xT packed [NQT, 128, DC, ST] bf16 st-major
        xb = np.asarray(x, np.float32).astype(bf)
        return np.ascontiguousarray(
            xb.T.reshape(DC, P, NQT, ST).transpose(2, 1, 0, 3)
        )

    def packX8(x):  # [S, D] fp32 -> [NQT, 128, DCP, 2, ST] e4m3 st-major
        xb = np.asarray(x, np.float32).T.astype(e4)  # [D, S]
        return np.ascontiguousarray(
            xb.reshape(DCP, 2, P, NQT, ST).transpose(3, 2, 0, 1, 4)
        )

    def packW8(w):  # [D, NHL] -> [128, DCP, 2, NHL] e4m3, pre-scaled x64
        wb = (np.asarray(w, np.float32) * WS).astype(e4)
        return np.ascontiguousarray(wb.reshape(DCP, 2, P, NHL).transpose(2, 0, 1, 3))

    def packW(w):  # [D, NHL] -> [128, DC, NHL]
        wb = np.asarray(w, np.float32).astype(bf)
        return np.ascontiguousarray(wb.reshape(DC, P, NHL).transpose(1, 0, 2))

    def packWo(w):  # [NHL, D] -> [128, MC, D]
        wb = np.asarray(w, np.float32).astype(bf)
        return np.ascontiguousarray(wb.reshape(MC, P, D).transpose(1, 0, 2))

    query = np.asarray(query, np.float32)
    key = np.asarray(key, np.float32)
    value = np.asarray(value, np.float32)
    in_maps = []
    for c in range(NCORES):
        b, hg = divmod(c, 2)
        cols = slice(hg * NHL, (hg + 1) * NHL)
        in_maps.append(
            {
                "xq8": packX8(query[b]),
                "xk8": packX8(key[b]),
                "xvT": packT(value[b]),
                "wq8": packW8(np.asarray(w_q)[:, cols]),
                "wk8": packW8(np.asarray(w_k)[:, cols]),
                "wv": packW(np.asarray(w_v)[:, cols]),
                "wo": packWo(np.asarray(w_o)[cols, :]),
            }
        )
    return in_maps


def kernel(query, key, value, mask, w_q, w_k, w_v, w_o, **run_kwargs):
    nc = build_nc()
    in_maps = make_in_maps(query, key, value, w_q, w_k, w_v, w_o)
    res = run_bass_kernel_spmd(nc, in_maps, list(range(NCORES)), **run_kwargs)
    out = np.empty((B, S, D), np.float32)
    for b in range(B):
        out[b] = res.results[2 * b]["out"] + res.results[2 * b + 1]["out"]
    return out



# revision 22
# speedup vs baseline: 1.3386x; 1.0181x over previous
"""Distributed causal MultiHeadAttention kernel for 8 Trainium2 NeuronCores.

Problem: B=4, S=2048, D=1024, H=16 heads, dk=dv=64, causal mask, fp32 I/O.

Sharding: data-parallel over batch (4) x tensor-parallel over heads (2 groups
of 8) = 8 cores. Core c handles batch c//2 with heads (c%2)*8 .. (c%2)*8+7.
Each core computes a partial output [S, D] (its head group's contribution
through the corresponding w_o rows); the host sums the pair of partials per
batch (the "all-reduce" of the output projection, done host-side).

Device dataflow (bf16 matmuls with fp32 PSUM accumulation, except q/k
projections which run fp8e4m3 in DoubleRow perf mode = 4x PE throughput;
w_q/w_k are pre-scaled x64 host-side and the 1/64^2 is folded into the
softmax exp scale, which keeps the fp8 quantization noise confined to the
logits where softmax normalization damps it; measured end-to-end absmax
rel err ~1.1% vs the 2% gate):

  - The whole schedule is organized so the ScalarE exp stream (the real
    bottleneck: ~139k elems/lane at 1.2 GHz ~= 130+ us) never starves:
    only the first 512-row tile of kT/qT/v is projected up front (~10 us
    incl. its input DMA, issued st-major), then attention starts and ALL
    remaining projection work + the per-q-tile output projections are
    emitted as small "fill units" interleaved one-per-k-chunk into the
    attention instruction stream, so the PE executes them inside the gaps
    of the ACT-bound attention phase instead of serializing in front of it.
  - qT/kT = w.T @ xT -> [512, S] head-major rows (fp8 DoubleRow chains).
  - v = xT.T @ wv -> [S, 512] with a constant 1.0 column per head
    ([S, 8, 65]) so A@V also produces softmax row sums ("ones trick").
  - Scores per head pair, transposed: S^T[k, q] = kT.T @ qT; the even/odd
    head rows sit at partitions 0-63 / 64-127 so the two dk=64 matmuls
    row-tile onto disjoint PE quadrants and run concurrently.
  - One exp per (pair, q-tile, k-chunk) on ScalarE straight out of PSUM
    (scale folded in; no max subtraction - scores are O(1) bounded).
    Causal mask: fully-masked column ranges are skipped outright, the
    triangular 128x128 diagonal block is zeroed post-exp via gpsimd
    affine_select.
  - out^T[dv(+1), q] accumulated over k-chunks: lhsT = [V_h | 1], rhs = A^T.
    Row 64 is the softmax denominator r[q]; per head pair the reciprocal is
    taken on DVE and broadcast across partitions with an SBUF->SBUF DMA.
  - Output projection per q-tile (oT.T @ wo) is queued as fill units right
    after that q-tile's normalization.
"""

import collections

import numpy as np
import ml_dtypes

import concourse.bass as bass
import concourse.bacc as bacc
import concourse.mybir as mybir
import concourse.tile as tile
from concourse.bass_utils import run_bass_kernel_spmd

B, S, D = 4, 2048, 1024
H, DK = 16, 64
HL = 8              # heads handled per core
NHL = HL * DK       # 512 rows of head-dim per core
P = 128
NCORES = 8
ST = 512            # q-tile width (matmul free dim / PSUM bank)
NQT = S // ST       # 4
NKC = S // P        # 16 k chunks
MC = NHL // P       # 4 head-pair chunks
DC = D // P         # 8 chunks of D
DCP = DC // 2       # 4 DoubleRow pair-chunks of D

FP32 = mybir.dt.float32
BF16 = mybir.dt.bfloat16
FP8 = mybir.dt.float8e4
EXP = mybir.ActivationFunctionType.Exp
DR = mybir.MatmulPerfMode.DoubleRow

WS = 64.0                        # host-side pre-scale on w_q / w_k
EXP_SCALE = 0.125 / (WS * WS)    # 1/sqrt(dk) / ws^2 = 2^-15, exact


def _emit(tc):
    nc = tc.nc

    # x inputs are st-major so each 512-column slice is one contiguous
    # 4-8 KB-per-partition DMA (512 B descriptors run ~8x slower)
    xq8 = nc.dram_tensor("xq8", [NQT, P, DCP, 2, ST], FP8, kind="ExternalInput").ap()
    xk8 = nc.dram_tensor("xk8", [NQT, P, DCP, 2, ST], FP8, kind="ExternalInput").ap()
    xvT = nc.dram_tensor("xvT", [NQT, P, DC, ST], BF16, kind="ExternalInput").ap()
    wq8 = nc.dram_tensor("wq8", [P, DCP, 2, NHL], FP8, kind="ExternalInput").ap()
    wk8 = nc.dram_tensor("wk8", [P, DCP, 2, NHL], FP8, kind="ExternalInput").ap()
    wv = nc.dram_tensor("wv", [P, DC, NHL], BF16, kind="ExternalInput").ap()
    wo = nc.dram_tensor("wo", [P, MC, D], BF16, kind="ExternalInput").ap()
    # bf16 partial outputs (summed in fp32 on the host) cost ~0.1% extra
    # error but halve the output DMA traffic the norm chains contend with
    out = nc.dram_tensor("out", [S, D], BF16, kind="ExternalOutput").ap()

    with (
        tc.tile_pool(name="sing", bufs=1) as sing,
        tc.tile_pool(name="apool", bufs=6) as apool,
        tc.tile_pool(name="rpool", bufs=2) as rpool,
        tc.tile_pool(name="outp", bufs=3) as outp,
        tc.tile_pool(name="dpool", bufs=3, space="DRAM") as dpool,
        tc.tile_pool(name="psS", bufs=2, space="PSUM") as psS,
        tc.tile_pool(name="psO", bufs=2, space="PSUM") as psO,
        tc.tile_pool(name="psP", bufs=2, space="PSUM") as psP,
    ):
        # ---- persistent SBUF tiles -------------------------------------
        wq_sb = sing.tile([P, DCP, 2, NHL], FP8, tag="wq_sb")
        wk_sb = sing.tile([P, DCP, 2, NHL], FP8, tag="wk_sb")
        wv_sb = sing.tile([P, DC, NHL], BF16, tag="wv_sb")
        wo_sb = sing.tile([P, MC, D], BF16, tag="wo_sb")
        xq_sb = sing.tile([P, NQT, DCP, 2, ST], FP8, tag="xq_sb")
        xk_sb = sing.tile([P, NQT, DCP, 2, ST], FP8, tag="xk_sb")
        xv_sb = sing.tile([P, NQT, DC, ST], BF16, tag="xv_sb")
        qT = sing.tile([P, MC, S], BF16, tag="qT")
        kT = sing.tile([P, MC, S], BF16, tag="kT")
        v65 = sing.tile([P, NKC, HL, DK + 1], BF16, tag="v65")
        oT = sing.tile([P, MC, S], BF16, tag="oT")

        nc.gpsimd.memset(v65[:, :, :, DK : DK + 1], 1.0)
        warm = sing.tile([P, P], BF16, tag="warm")
        nc.gpsimd.memset(warm, 0.0)

        # ---- DMA issue order = first-need order (st-major for x).
        # Two queues: the q/k path on the SP ring, the v path + wo on the
        # Pool ring, so xv st0 lands in time for the JIT v-projection fill
        # units instead of queueing behind 2 MB of q/k input.
        # the k-path loads on the SP ring and the q-path on the Pool ring so
        # the kT-st0 and qT-st0 projections become DMA-ready in parallel
        # (~16us) instead of serially (~27us for the q side).
        nc.sync.dma_start(wk_sb, wk8)
        nc.sync.dma_start(xk_sb[:, 0], xk8[0])
        nc.gpsimd.dma_start(wq_sb, wq8)
        nc.gpsimd.dma_start(xq_sb[:, 0], xq8[0])
        nc.gpsimd.dma_start(wv_sb, wv)
        for st in range(NQT):
            nc.gpsimd.dma_start(xv_sb[:, st], xvT[st])
        nc.gpsimd.dma_start(wo_sb, wo)
        for st in range(1, NQT):
            nc.sync.dma_start(xk_sb[:, st], xk8[st])
            nc.sync.dma_start(xq_sb[:, st], xq8[st])

        # ~6us of dummy matmuls while the first input DMAs land: trips the
        # PE HAM activity window to K=8/8 so the real stream starts at
        # 2.4 GHz instead of paying the cold 1.2 GHz ramp.
        wps = psP.tile([P, P], FP32, tag="psP")
        for _ in range(112):
            nc.tensor.matmul(wps[0:P, 0:64], lhsT=warm[:, 0:P], rhs=warm[:, 0:64],
                             start=True, stop=True)

        # ---- projection units ------------------------------------------
        def proj_qk(w_sb, x_sb, dst, st, mc):
            ps = psP.tile([P, ST], FP32, tag="psP")
            for dcp in range(DCP):
                nc.tensor.matmul(
                    ps,
                    lhsT=w_sb[:, dcp, :, mc * P : (mc + 1) * P],
                    rhs=x_sb[:, st, dcp, :, :],
                    start=(dcp == 0),
                    stop=(dcp == DCP - 1),
                    perf_mode=DR,
                )
            nc.vector.tensor_copy(dst[:, mc, st * ST : (st + 1) * ST], ps)

        def proj_v(sc):
            st, j = divmod(sc, ST // P)
            ps = psP.tile([P, ST], FP32, tag="psP")
            for dc in range(DC):
                nc.tensor.matmul(
                    ps,
                    lhsT=xv_sb[:, st, dc, j * P : (j + 1) * P],
                    rhs=wv_sb[:, dc, :],
                    start=(dc == 0),
                    stop=(dc == DC - 1),
                )
            nc.vector.tensor_copy(
                v65[:, sc, :, 0:DK], ps.rearrange("p (h d) -> p h d", h=HL)
            )

        def o_proj(sc, nt):
            ps = psP.tile([P, ST], FP32, tag="psP")
            for c in range(MC):
                nc.tensor.matmul(
                    ps,
                    lhsT=oT[:, c, sc * P : (sc + 1) * P],
                    rhs=wo_sb[:, c, nt * ST : (nt + 1) * ST],
                    start=(c == 0),
                    stop=(c == MC - 1),
                )
            ob = outp.tile([P, ST], BF16, tag="ob")
            nc.vector.tensor_copy(ob, ps)
            nc.sync.dma_start(
                out[sc * P : (sc + 1) * P, nt * ST : (nt + 1) * ST], ob
            )

        # preamble: only what q-tile 0's first scores need
        for mc in range(MC):
            proj_qk(wk_sb, xk_sb, kT, 0, mc)
        for mc in range(MC):
            proj_qk(wq_sb, xq_sb, qT, 0, mc)

        # deferred PE work, drip-fed into the ACT-bound attention stream.
        # v chunks 0-3 first: A@V for (qt0, kc) needs v chunk kc, and the
        # first A@V is emitted one step after the first exp. qT st1 before
        # kT st1: q-tile 1's first scores read qT st1, while kT st1 isn't
        # touched until its 5th k-chunk.
        # CAUTION: a unit must be POPPED (emitted) before the first attention
        # instruction that reads its output is emitted — emission order is
        # what the dependency tracker sees; a consumer emitted first reads
        # stale data with no ordering at all.
        fill = collections.deque()
        for sc in range(ST // P):
            fill.append((lambda sc=sc: proj_v(sc)))
        for sc in range(ST // P, 2 * (ST // P)):
            fill.append((lambda sc=sc: proj_v(sc)))
        for mc in range(MC):
            fill.append((lambda mc=mc: proj_qk(wq_sb, xq_sb, qT, 1, mc)))
        for mc in range(MC):
            fill.append((lambda mc=mc: proj_qk(wk_sb, xk_sb, kT, 1, mc)))
        for st in range(2, NQT):
            for mc in range(MC):
                fill.append((lambda st=st, mc=mc: proj_qk(wq_sb, xq_sb, qT, st, mc)))
            for mc in range(MC):
                fill.append((lambda st=st, mc=mc: proj_qk(wk_sb, xk_sb, kT, st, mc)))
            for sc in range(st * (ST // P), (st + 1) * (ST // P)):
                fill.append((lambda sc=sc: proj_v(sc)))

        tick = [0]

        def pump(n=1):
            for _ in range(n):
                if fill:
                    fill.popleft()()

        pump(2)  # v chunks 0-1 ahead of the first A@V

        # ---- attention + interleaved fill --------------------------------
        def clo(kc, qt):
            j = kc - qt * (ST // P)
            return j * P if j > 0 else 0

        def emit_av(a_t, kc, oT_ps, pc, qt, nkc):
            lo = clo(kc, qt)
            for hh in range(2):
                nc.tensor.matmul(
                    oT_ps[hh][:, lo:ST],
                    lhsT=v65[:, kc, 2 * pc + hh, :],
                    rhs=a_t[:, hh, lo:ST],
                    start=(kc == 0),
                    stop=(kc == nkc - 1),
                )

        pending = []      # previous q-tile's o_proj units
        reserve = []      # units held back to cover the final norm window
        norm_mul = []     # deferred normalizing multiply (see below)
        for qt in range(NQT):
            for pc in range(MC):
                # release the previous q-tile's output projection only one
                # bucket in, after its normalization has landed, so those
                # units never head-of-line-block the PE queue. For the last
                # q-tile, half the units are held in reserve to keep the PE
                # fed while the final pc's normalization chain completes.
                if pc == 1 and pending:
                    if qt == NQT - 1:
                        fill.extend(pending[: len(pending) // 2])
                        reserve = pending[len(pending) // 2 :]
                    else:
                        fill.extend(pending)
                    pending = []
                nkc = (qt + 1) * (ST // P)
                oT_ps = [
                    psO.tile([DK + 1, ST], FP32, tag="psO", name=f"psO_{hh}")
                    for hh in range(2)
                ]
                prev = []
                for kc in range(nkc):
                    lo = clo(kc, qt)
                    sps = psS.tile([P, 2, ST], FP32, tag="psS", name="sps")
                    for hh in range(2):
                        pp = hh * 64
                        nc.tensor.matmul(
                            sps[:, hh, lo:ST],
                            lhsT=kT[pp : pp + 64, pc, kc * P : (kc + 1) * P],
                            rhs=qT[pp : pp + 64, pc, qt * ST + lo : (qt + 1) * ST],
                            start=True,
                            stop=True,
                        )
                    a_t = apool.tile([P, 2, ST], BF16, tag="a", name="a_t")
                    nc.scalar.activation(
                        a_t[:, :, lo:ST], sps[:, :, lo:ST], EXP,
                        bias=0.0, scale=EXP_SCALE,
                    )
                    if kc >= qt * (ST // P):
                        # triangular block: keep where q_local >= k_local
                        nc.gpsimd.affine_select(
                            out=a_t[:, :, lo : lo + P],
                            in_=a_t[:, :, lo : lo + P],
                            pattern=[[0, 2], [1, P]],
                            channel_multiplier=-1,
                            base=0,
                            compare_op=mybir.AluOpType.is_ge,
                            fill=0.0,
                        )
                    prev.append((a_t, kc))
                    if len(prev) > 1:
                        a_p, kc_p = prev.pop(0)
                        emit_av(a_p, kc_p, oT_ps, pc, qt, nkc)
                    # pacing: q-tile 0 pumps every step so all 16 st0/st1
                    # units are emitted before q-tile 1's consumers; later
                    # q-tiles pump one unit per two k-chunks, matching the
                    # per-step ACT slack so the exp stream stays packed
                    tick[0] += 1
                    if qt == 0 or tick[0] % 2 == 0:
                        pump(1)
                for a_p, kc_p in prev:
                    emit_av(a_p, kc_p, oT_ps, pc, qt, nkc)

                # the previous pc's normalizing multiply is emitted only now,
                # after this pc's diagonal-mask affine_selects: the Pool
                # queue is strict FIFO, and the multiply waits on its rrep
                # broadcast DMA, so emitting it earlier would head-of-line-
                # block those affine_selects (which gate A@V, i.e. the PE).
                # By now its broadcast has had a whole kc loop to land.
                if norm_mul:
                    norm_mul.pop(0)()

                # per-pc normalization: copy the raw (unnormalized) oT out,
                # reciprocal the two softmax-sum rows lane-parallel ([128, 8]
                # via a DRAM reshape - a flat [1, 512] DVE reciprocal costs
                # 3.3us), broadcast across partitions from DRAM. The chain is
                # off the PE path and hides behind the next pc's attention,
                # so no q-tile boundary ever stalls the PE.
                # the final pc's chain rides the ACT ring (idle at the tail,
                # the exp stream is long finished) so it never queues behind
                # in-flight output DMAs on the SP ring
                eng = nc.scalar if (qt == NQT - 1 and pc == MC - 1) else nc.sync
                rd_q = dpool.tile([2, ST], FP32, tag="rdq", name="rd_q")
                for hh in range(2):
                    ps = oT_ps[hh]
                    rsb = rpool.tile([1, ST], FP32, tag=f"rsb{hh}")
                    nc.vector.tensor_copy(rsb, ps[DK : DK + 1, :])
                    nc.vector.tensor_copy(
                        oT[hh * 64 : (hh + 1) * 64, pc, qt * ST : (qt + 1) * ST],
                        ps[0:DK, :],
                    )
                    eng.dma_start(rd_q[hh : hh + 1, :], rsb)
                r128 = rpool.tile([P, 2 * ST // P], FP32, tag="r128")
                eng.dma_start(r128, rd_q.rearrange("a (p f) -> (a p) f", p=64))
                ri128 = rpool.tile([P, 2 * ST // P], FP32, tag="ri128")
                nc.vector.reciprocal(ri128, r128)
                rd_i = dpool.tile([2, ST], FP32, tag="rdi", name="rd_i")
                eng.dma_start(rd_i.rearrange("a (p f) -> (a p) f", p=64), ri128)
                rrep = rpool.tile([P, ST], FP32, tag="rrep")
                for hh in range(2):
                    eng.dma_start(
                        rrep[hh * 64 : (hh + 1) * 64, :],
                        rd_i[hh : hh + 1, :].to_broadcast((64, ST)),
                    )
                sl = oT[:, pc, qt * ST : (qt + 1) * ST]
                norm_mul.append(
                    lambda sl=sl, rrep=rrep: nc.gpsimd.tensor_mul(sl, sl, rrep)
                )

            # hold this q-tile's output projection back until one bucket
            # into the next q-tile (see `pending` release above)
            pending = [
                (lambda sc=sc, nt=nt: o_proj(sc, nt))
                for sc in range(qt * (ST // P), (qt + 1) * (ST // P))
                for nt in range(D // ST)
            ]

        # tail: emit the last pc's multiply (still waiting on its broadcast),
        # then the reserved units (normalized long ago) keep the PE busy
        # while that chain completes, then the final q-tile's own output
        # projections drain.
        if norm_mul:
            norm_mul.pop(0)()
        fill.extend(reserve)
        fill.extend(pending)
        pump(len(fill))


_CACHE = {}


def build_nc():
    if "nc" not in _CACHE:
        # Bacc (not plain Bass): its finalize runs the pass pipeline that
        # splits multi-semaphore waits into event-semaphore/ldweights slots,
        # which walrus requires (max 1 wait per instruction on TRN2).
        nc = bacc.Bacc()
        with tile.TileContext(nc) as tc:
            _emit(tc)
        nc.finalize()
        _CACHE["nc"] = nc
    return _CACHE["nc"]


def make_in_maps(query, key, value, w_q, w_k, w_v, w_o):
    bf = ml_dtypes.bfloat16
    e4 = ml_dtypes.float8_e4m3

    def packT(x):  # [S, D] fp32 -> xT packed [NQT, 128, DC, ST] bf16 st-major
        xb = np.asarray(x, np.float32).astype(bf)
        return np.ascontiguousarray(
            xb.T.reshape(DC, P, NQT, ST).transpose(2, 1, 0, 3)
        )

    def packX8(x):  # [S, D] fp32 -> [NQT, 128, DCP, 2, ST] e4m3 st-major
        xb = np.asarray(x, np.float32).T.astype(e4)  # [D, S]
        return np.ascontiguousarray(
            xb.reshape(DCP, 2, P, NQT, ST).transpose(3, 2, 0, 1, 4)
        )

    def packW8(w):  # [D, NHL] -> [128, DCP, 2, NHL] e4m3, pre-scaled x64
        wb = (np.asarray(w, np.float32) * WS).astype(e4)
        return np.ascontiguousarray(wb.reshape(DCP, 2, P, NHL).transpose(2, 0, 1, 3))

    def packW(w):  # [D, NHL] -> [128, DC, NHL]
        wb = np.asarray(w, np.float32).astype(bf)
        return np.ascontiguousarray(wb.reshape(DC, P, NHL).transpose(1, 0, 2))

    def packWo(w):  # [NHL, D] -> [128, MC, D]
        wb = np.asarray(w, np.float32).astype(bf)
        return np.ascontiguousarray(wb.reshape(MC, P, D).transpose(1, 0, 2))

    query = np.asarray(query, np.float32)
    key = np.asarray(key, np.float32)
    value = np.asarray(value, np.float32)
    in_maps = []
    for c in range(NCORES):
        b, hg = divmod(c, 2)
        cols = slice(hg * NHL, (hg + 1) * NHL)
        in_maps.append(
            {
                "xq8": packX8(query[b]),
                "xk8": packX8(key[b]),
                "xvT": packT(value[b]),
                "wq8": packW8(np.asarray(w_q)[:, cols]),
                "wk8": packW8(np.asarray(w_k)[:, cols]),
                "wv": packW(np.asarray(w_v)[:, cols]),
                "wo": packWo(np.asarray(w_o)[cols, :]),
            }
        )
    return in_maps


def kernel(query, key, value, mask, w_q, w_k, w_v, w_o, **run_kwargs):
    nc = build_nc()
    in_maps = make_in_maps(query, key, value, w_q, w_k, w_v, w_o)
    res = run_bass_kernel_spmd(nc, in_maps, list(range(NCORES)), **run_kwargs)
    out = np.empty((B, S, D), np.float32)
    for b in range(B):
        out[b] = np.asarray(res.results[2 * b]["out"], np.float32) + np.asarray(
            res.results[2 * b + 1]["out"], np.float32
        )
    return out



# revision 28
# speedup vs baseline: 1.3767x; 1.0284x over previous
"""Distributed causal MultiHeadAttention kernel for 8 Trainium2 NeuronCores.

Problem: B=4, S=2048, D=1024, H=16 heads, dk=dv=64, causal mask, fp32 I/O.

Sharding: data-parallel over batch (4) x tensor-parallel over heads (2 groups
of 8) = 8 cores. Core c handles batch c//2 with heads (c%2)*8 .. (c%2)*8+7.
Each core computes a partial output [S, D] (its head group's contribution
through the corresponding w_o rows); the host sums the pair of partials per
batch (the "all-reduce" of the output projection, done host-side).

Device dataflow (bf16 matmuls with fp32 PSUM accumulation, except q/k
projections which run fp8e4m3 in DoubleRow perf mode = 4x PE throughput;
w_q/w_k are pre-scaled x64 host-side and the 1/64^2 is folded into the
softmax exp scale, which keeps the fp8 quantization noise confined to the
logits where softmax normalization damps it; measured end-to-end absmax
rel err ~1.1% vs the 2% gate):

  - The whole schedule is organized so the ScalarE exp stream (the real
    bottleneck: ~139k elems/lane at 1.2 GHz ~= 130+ us) never starves:
    only the first 512-row tile of kT/qT/v is projected up front (~10 us
    incl. its input DMA, issued st-major), then attention starts and ALL
    remaining projection work + the per-q-tile output projections are
    emitted as small "fill units" interleaved one-per-k-chunk into the
    attention instruction stream, so the PE executes them inside the gaps
    of the ACT-bound attention phase instead of serializing in front of it.
  - qT/kT = w.T @ xT -> [512, S] head-major rows (fp8 DoubleRow chains).
  - v = xT.T @ wv -> [S, 512] with a constant 1.0 column per head
    ([S, 8, 65]) so A@V also produces softmax row sums ("ones trick").
  - Scores per head pair, transposed: S^T[k, q] = kT.T @ qT; the even/odd
    head rows sit at partitions 0-63 / 64-127 so the two dk=64 matmuls
    row-tile onto disjoint PE quadrants and run concurrently.
  - One exp per (pair, q-tile, k-chunk) on ScalarE straight out of PSUM
    (scale folded in; no max subtraction - scores are O(1) bounded).
    Causal mask: fully-masked column ranges are skipped outright, the
    triangular 128x128 diagonal block is zeroed post-exp via gpsimd
    affine_select.
  - out^T[dv(+1), q] accumulated over k-chunks: lhsT = [V_h | 1], rhs = A^T.
    Row 64 is the softmax denominator r[q]. Normalization is per head pair
    (per pc): the two sum rows bounce through DRAM into a lane-parallel
    [128, 8] reciprocal, broadcast back across partitions from DRAM, and a
    Pool-engine multiply - whose EMISSION is deferred one pc so its DMA wait
    never head-of-line-blocks the affine_selects in the strict-FIFO Pool
    queue. The very last pc instead uses a DMA-free chain (packed [1, 1024]
    approx-reciprocal + K=1 ones-matmul partition broadcast + DVE multiply)
    so the tail only exposes ~3us, covered by reserved o_proj fill units.
  - Output projection per q-tile (oT.T @ wo) is queued as fill units one
    bucket into the next q-tile; partial outputs leave as bf16 (summed in
    fp32 on the host), halving output DMA traffic.
  - ~6us of dummy N=64 matmuls at the start trip the PE HAM activity window
    to K=8/8 before the first projection; q-path input DMAs ride the Pool
    ring and k-path the SP ring so both st0 projections are ready ~16us.
"""

import collections

import numpy as np
import ml_dtypes

import concourse.bass as bass
import concourse.bacc as bacc
import concourse.mybir as mybir
import concourse.tile as tile
from concourse.bass_utils import run_bass_kernel_spmd

B, S, D = 4, 2048, 1024
H, DK = 16, 64
HL = 8              # heads handled per core
NHL = HL * DK       # 512 rows of head-dim per core
P = 128
NCORES = 8
ST = 512            # q-tile width (matmul free dim / PSUM bank)
NQT = S // ST       # 4
NKC = S // P        # 16 k chunks
MC = NHL // P       # 4 head-pair chunks
DC = D // P         # 8 chunks of D
DCP = DC // 2       # 4 DoubleRow pair-chunks of D

FP32 = mybir.dt.float32
BF16 = mybir.dt.bfloat16
FP8 = mybir.dt.float8e4
EXP = mybir.ActivationFunctionType.Exp
DR = mybir.MatmulPerfMode.DoubleRow

WS = 64.0                        # host-side pre-scale on w_q / w_k
EXP_SCALE = 0.125 / (WS * WS)    # 1/sqrt(dk) / ws^2 = 2^-15, exact


def _emit(tc):
    nc = tc.nc

    # x inputs are st-major so each 512-column slice is one contiguous
    # 4-8 KB-per-partition DMA (512 B descriptors run ~8x slower)
    xq8 = nc.dram_tensor("xq8", [NQT, P, DCP, 2, ST], FP8, kind="ExternalInput").ap()
    xk8 = nc.dram_tensor("xk8", [NQT, P, DCP, 2, ST], FP8, kind="ExternalInput").ap()
    xvT = nc.dram_tensor("xvT", [NQT, P, DC, ST], BF16, kind="ExternalInput").ap()
    wq8 = nc.dram_tensor("wq8", [P, DCP, 2, NHL], FP8, kind="ExternalInput").ap()
    wk8 = nc.dram_tensor("wk8", [P, DCP, 2, NHL], FP8, kind="ExternalInput").ap()
    wv = nc.dram_tensor("wv", [P, DC, NHL], BF16, kind="ExternalInput").ap()
    wo = nc.dram_tensor("wo", [P, MC, D], BF16, kind="ExternalInput").ap()
    # bf16 partial outputs (summed in fp32 on the host) cost ~0.1% extra
    # error but halve the output DMA traffic the norm chains contend with
    out = nc.dram_tensor("out", [S, D], BF16, kind="ExternalOutput").ap()

    with (
        tc.tile_pool(name="sing", bufs=1) as sing,
        tc.tile_pool(name="apool", bufs=6) as apool,
        tc.tile_pool(name="rpool", bufs=2) as rpool,
        tc.tile_pool(name="tailp", bufs=1) as tailp,
        tc.tile_pool(name="outp", bufs=3) as outp,
        tc.tile_pool(name="dpool", bufs=3, space="DRAM") as dpool,
        tc.tile_pool(name="psS", bufs=2, space="PSUM") as psS,
        tc.tile_pool(name="psO", bufs=2, space="PSUM") as psO,
        tc.tile_pool(name="psP", bufs=2, space="PSUM") as psP,
    ):
        # ---- persistent SBUF tiles -------------------------------------
        wq_sb = sing.tile([P, DCP, 2, NHL], FP8, tag="wq_sb")
        wk_sb = sing.tile([P, DCP, 2, NHL], FP8, tag="wk_sb")
        wv_sb = sing.tile([P, DC, NHL], BF16, tag="wv_sb")
        wo_sb = sing.tile([P, MC, D], BF16, tag="wo_sb")
        xq_sb = sing.tile([P, NQT, DCP, 2, ST], FP8, tag="xq_sb")
        xk_sb = sing.tile([P, NQT, DCP, 2, ST], FP8, tag="xk_sb")
        xv_sb = sing.tile([P, NQT, DC, ST], BF16, tag="xv_sb")
        qT = sing.tile([P, MC, S], BF16, tag="qT")
        kT = sing.tile([P, MC, S], BF16, tag="kT")
        v65 = sing.tile([P, NKC, HL, DK + 1], BF16, tag="v65")
        oT = sing.tile([P, MC, S], BF16, tag="oT")

        nc.gpsimd.memset(v65[:, :, :, DK : DK + 1], 1.0)
        warm = sing.tile([P, P], BF16, tag="warm")
        nc.gpsimd.memset(warm, 0.0)
        ones_t = sing.tile([1, P], FP32, tag="ones_t")
        nc.gpsimd.memset(ones_t, 1.0)

        # ---- DMA issue order = first-need order (st-major for x).
        # Two queues: the q/k path on the SP ring, the v path + wo on the
        # Pool ring, so xv st0 lands in time for the JIT v-projection fill
        # units instead of queueing behind 2 MB of q/k input.
        # the k-path loads on the SP ring and the q-path on the Pool ring so
        # the kT-st0 and qT-st0 projections become DMA-ready in parallel
        # (~16us) instead of serially (~27us for the q side).
        nc.sync.dma_start(wk_sb, wk8)
        nc.sync.dma_start(xk_sb[:, 0], xk8[0])
        nc.gpsimd.dma_start(wq_sb, wq8)
        nc.gpsimd.dma_start(xq_sb[:, 0], xq8[0])
        nc.gpsimd.dma_start(wv_sb, wv)
        for st in range(NQT):
            nc.gpsimd.dma_start(xv_sb[:, st], xvT[st])
        nc.gpsimd.dma_start(wo_sb, wo)
        for st in range(1, NQT):
            nc.sync.dma_start(xk_sb[:, st], xk8[st])
            nc.sync.dma_start(xq_sb[:, st], xq8[st])

        # ~6us of dummy matmuls while the first input DMAs land: trips the
        # PE HAM activity window to K=8/8 so the real stream starts at
        # 2.4 GHz instead of paying the cold 1.2 GHz ramp.
        wps = psP.tile([P, P], FP32, tag="psP")
        for _ in range(112):
            nc.tensor.matmul(wps[0:P, 0:64], lhsT=warm[:, 0:P], rhs=warm[:, 0:64],
                             start=True, stop=True)

        # ---- projection units ------------------------------------------
        def proj_qk(w_sb, x_sb, dst, st, mc):
            ps = psP.tile([P, ST], FP32, tag="psP")
            for dcp in range(DCP):
                nc.tensor.matmul(
                    ps,
                    lhsT=w_sb[:, dcp, :, mc * P : (mc + 1) * P],
                    rhs=x_sb[:, st, dcp, :, :],
                    start=(dcp == 0),
                    stop=(dcp == DCP - 1),
                    perf_mode=DR,
                )
            nc.vector.tensor_copy(dst[:, mc, st * ST : (st + 1) * ST], ps)

        def proj_v(sc):
            st, j = divmod(sc, ST // P)
            ps = psP.tile([P, ST], FP32, tag="psP")
            for dc in range(DC):
                nc.tensor.matmul(
                    ps,
                    lhsT=xv_sb[:, st, dc, j * P : (j + 1) * P],
                    rhs=wv_sb[:, dc, :],
                    start=(dc == 0),
                    stop=(dc == DC - 1),
                )
            nc.vector.tensor_copy(
                v65[:, sc, :, 0:DK], ps.rearrange("p (h d) -> p h d", h=HL)
            )

        def o_proj(sc, nt):
            ps = psP.tile([P, ST], FP32, tag="psP")
            for c in range(MC):
                nc.tensor.matmul(
                    ps,
                    lhsT=oT[:, c, sc * P : (sc + 1) * P],
                    rhs=wo_sb[:, c, nt * ST : (nt + 1) * ST],
                    start=(c == 0),
                    stop=(c == MC - 1),
                )
            ob = outp.tile([P, ST], BF16, tag="ob")
            nc.vector.tensor_copy(ob, ps)
            nc.sync.dma_start(
                out[sc * P : (sc + 1) * P, nt * ST : (nt + 1) * ST], ob
            )

        # preamble: only what q-tile 0's first scores need
        for mc in range(MC):
            proj_qk(wk_sb, xk_sb, kT, 0, mc)
        for mc in range(MC):
            proj_qk(wq_sb, xq_sb, qT, 0, mc)

        # deferred PE work, drip-fed into the ACT-bound attention stream.
        # v chunks 0-3 first: A@V for (qt0, kc) needs v chunk kc, and the
        # first A@V is emitted one step after the first exp. qT st1 before
        # kT st1: q-tile 1's first scores read qT st1, while kT st1 isn't
        # touched until its 5th k-chunk.
        # CAUTION: a unit must be POPPED (emitted) before the first attention
        # instruction that reads its output is emitted — emission order is
        # what the dependency tracker sees; a consumer emitted first reads
        # stale data with no ordering at all.
        fill = collections.deque()
        for sc in range(ST // P):
            fill.append((lambda sc=sc: proj_v(sc)))
        for sc in range(ST // P, 2 * (ST // P)):
            fill.append((lambda sc=sc: proj_v(sc)))
        for mc in range(MC):
            fill.append((lambda mc=mc: proj_qk(wq_sb, xq_sb, qT, 1, mc)))
        for mc in range(MC):
            fill.append((lambda mc=mc: proj_qk(wk_sb, xk_sb, kT, 1, mc)))
        for st in range(2, NQT):
            for mc in range(MC):
                fill.append((lambda st=st, mc=mc: proj_qk(wq_sb, xq_sb, qT, st, mc)))
            for mc in range(MC):
                fill.append((lambda st=st, mc=mc: proj_qk(wk_sb, xk_sb, kT, st, mc)))
            for sc in range(st * (ST // P), (st + 1) * (ST // P)):
                fill.append((lambda sc=sc: proj_v(sc)))

        tick = [0]

        def pump(n=1):
            for _ in range(n):
                if fill:
                    fill.popleft()()

        pump(2)  # v chunks 0-1 ahead of the first A@V

        # ---- attention + interleaved fill --------------------------------
        def clo(kc, qt):
            j = kc - qt * (ST // P)
            return j * P if j > 0 else 0

        def emit_av(a_t, kc, oT_ps, pc, qt, nkc):
            lo = clo(kc, qt)
            for hh in range(2):
                nc.tensor.matmul(
                    oT_ps[hh][:, lo:ST],
                    lhsT=v65[:, kc, 2 * pc + hh, :],
                    rhs=a_t[:, hh, lo:ST],
                    start=(kc == 0),
                    stop=(kc == nkc - 1),
                )

        pending = []      # previous q-tile's o_proj units
        reserve = []      # units held back to cover the final norm window
        norm_mul = []     # deferred normalizing multiply (see below)
        finish_tail = []  # deferred tail broadcast + normalize
        for qt in range(NQT):
            for pc in range(MC):
                # release the previous q-tile's output projection only one
                # bucket in, after its normalization has landed, so those
                # units never head-of-line-block the PE queue. For the last
                # q-tile, half the units are held in reserve to keep the PE
                # fed while the final pc's normalization chain completes.
                if pc == 1 and pending:
                    if qt == NQT - 1:
                        fill.extend(pending[: len(pending) // 2])
                        reserve = pending[len(pending) // 2 :]
                    else:
                        fill.extend(pending)
                    pending = []
                nkc = (qt + 1) * (ST // P)
                oT_ps = [
                    psO.tile([DK + 1, ST], FP32, tag="psO", name=f"psO_{hh}")
                    for hh in range(2)
                ]
                prev = []
                for kc in range(nkc):
                    lo = clo(kc, qt)
                    sps = psS.tile([P, 2, ST], FP32, tag="psS", name="sps")
                    for hh in range(2):
                        pp = hh * 64
                        nc.tensor.matmul(
                            sps[:, hh, lo:ST],
                            lhsT=kT[pp : pp + 64, pc, kc * P : (kc + 1) * P],
                            rhs=qT[pp : pp + 64, pc, qt * ST + lo : (qt + 1) * ST],
                            start=True,
                            stop=True,
                        )
                    a_t = apool.tile([P, 2, ST], BF16, tag="a", name="a_t")
                    nc.scalar.activation(
                        a_t[:, :, lo:ST], sps[:, :, lo:ST], EXP,
                        bias=0.0, scale=EXP_SCALE,
                    )
                    if kc >= qt * (ST // P):
                        # triangular block: keep where q_local >= k_local
                        nc.gpsimd.affine_select(
                            out=a_t[:, :, lo : lo + P],
                            in_=a_t[:, :, lo : lo + P],
                            pattern=[[0, 2], [1, P]],
                            channel_multiplier=-1,
                            base=0,
                            compare_op=mybir.AluOpType.is_ge,
                            fill=0.0,
                        )
                    prev.append((a_t, kc))
                    if len(prev) > 1:
                        a_p, kc_p = prev.pop(0)
                        emit_av(a_p, kc_p, oT_ps, pc, qt, nkc)
                    # pacing: q-tile 0 pumps every step so all 16 st0/st1
                    # units are emitted before q-tile 1's consumers; later
                    # q-tiles pump one unit per two k-chunks, matching the
                    # per-step ACT slack so the exp stream stays packed
                    tick[0] += 1
                    if qt == 0 or tick[0] % 2 == 0:
                        pump(1)
                for a_p, kc_p in prev:
                    emit_av(a_p, kc_p, oT_ps, pc, qt, nkc)

                # the previous pc's normalizing multiply is emitted only now,
                # after this pc's diagonal-mask affine_selects: the Pool
                # queue is strict FIFO, and the multiply waits on its rrep
                # broadcast DMA, so emitting it earlier would head-of-line-
                # block those affine_selects (which gate A@V, i.e. the PE).
                # By now its broadcast has had a whole kc loop to land.
                if norm_mul:
                    norm_mul.pop(0)()

                # per-pc normalization: copy the raw (unnormalized) oT out,
                # reciprocal the two softmax-sum rows lane-parallel ([128, 8]
                # via a DRAM reshape - a flat [1, 512] DVE reciprocal costs
                # 3.3us), broadcast across partitions from DRAM. The chain is
                # off the PE path and hides behind the next pc's attention,
                # so no q-tile boundary ever stalls the PE.
                sl = oT[:, pc, qt * ST : (qt + 1) * ST]
                if qt == NQT - 1 and pc == MC - 1:
                    # tail fast path, no DMA hops: pack both softmax-sum rows
                    # into one [1, 1024] tile, approx-reciprocal (18-bit,
                    # plenty for a softmax denominator), then broadcast
                    # across partitions with a K=1 ones-matmul into PSUM and
                    # normalize on DVE. finish_tail is emitted only after the
                    # reserved o_proj units so they cover the reciprocal
                    # latency on the PE.
                    rsb_t = tailp.tile([1, 2 * ST], FP32, tag="rsbt")
                    for hh in range(2):
                        ps = oT_ps[hh]
                        nc.vector.tensor_copy(
                            rsb_t[0:1, hh * ST : (hh + 1) * ST], ps[DK : DK + 1, :]
                        )
                        nc.vector.tensor_copy(
                            oT[hh * 64 : (hh + 1) * 64, pc, qt * ST : (qt + 1) * ST],
                            ps[0:DK, :],
                        )
                    ri_t = tailp.tile([1, 2 * ST], FP32, tag="rit")
                    nc.vector.reciprocal_approx_fast(ri_t, rsb_t)
                    rps = psS.tile([P, 2, ST], FP32, tag="psS", name="rps")

                    def _finish(rps=rps, ri_t=ri_t, sl=sl):
                        for hh in range(2):
                            nc.tensor.matmul(
                                rps[hh * 64 : (hh + 1) * 64, 0, :],
                                lhsT=ones_t[0:1, 0:64],
                                rhs=ri_t[0:1, hh * ST : (hh + 1) * ST],
                                start=True,
                                stop=True,
                                **({"tile_position": (0, 64)} if hh else {}),
                            )
                        nc.vector.tensor_mul(sl, sl, rps[:, 0, :])

                    finish_tail.append(_finish)
                else:
                    rd_q = dpool.tile([2, ST], FP32, tag="rdq", name="rd_q")
                    for hh in range(2):
                        ps = oT_ps[hh]
                        rsb = rpool.tile([1, ST], FP32, tag=f"rsb{hh}")
                        nc.vector.tensor_copy(rsb, ps[DK : DK + 1, :])
                        nc.vector.tensor_copy(
                            oT[hh * 64 : (hh + 1) * 64, pc, qt * ST : (qt + 1) * ST],
                            ps[0:DK, :],
                        )
                        nc.sync.dma_start(rd_q[hh : hh + 1, :], rsb)
                    r128 = rpool.tile([P, 2 * ST // P], FP32, tag="r128")
                    nc.sync.dma_start(r128, rd_q.rearrange("a (p f) -> (a p) f", p=64))
                    ri128 = rpool.tile([P, 2 * ST // P], FP32, tag="ri128")
                    nc.vector.reciprocal(ri128, r128)
                    rd_i = dpool.tile([2, ST], FP32, tag="rdi", name="rd_i")
                    nc.sync.dma_start(
                        rd_i.rearrange("a (p f) -> (a p) f", p=64), ri128
                    )
                    rrep = rpool.tile([P, ST], FP32, tag="rrep")
                    for hh in range(2):
                        nc.sync.dma_start(
                            rrep[hh * 64 : (hh + 1) * 64, :],
                            rd_i[hh : hh + 1, :].to_broadcast((64, ST)),
                        )
                    norm_mul.append(
                        lambda sl=sl, rrep=rrep: nc.gpsimd.tensor_mul(sl, sl, rrep)
                    )

            # hold this q-tile's output projection back until one bucket
            # into the next q-tile (see `pending` release above)
            pending = [
                (lambda sc=sc, nt=nt: o_proj(sc, nt))
                for sc in range(qt * (ST // P), (qt + 1) * (ST // P))
                for nt in range(D // ST)
            ]

        # tail: the reserved units (normalized long ago) keep the PE busy
        # while the last pc's reciprocal completes; then the broadcast +
        # normalize lands and the final q-tile's output projections drain.
        fill.extend(reserve)
        pump(len(fill))
        if finish_tail:
            finish_tail.pop(0)()
        fill.extend(pending)
        pump(len(fill))


_CACHE = {}


def build_nc():
    if "nc" not in _CACHE:
        # Bacc (not plain Bass): its finalize runs the pass pipeline that
        # splits multi-semaphore waits into event-semaphore/ldweights slots,
        # which walrus requires (max 1 wait per instruction on TRN2).
        nc = bacc.Bacc()
        with tile.TileContext(nc) as tc:
            _emit(tc)
        nc.finalize()
        _CACHE["nc"] = nc
    return _CACHE["nc"]


def make_in_maps(query, key, value, w_q, w_k, w_v, w_o):
    bf = ml_dtypes.bfloat16
    e4 = ml_dtypes.float8_e4m3

    def packT(x):  # [S, D] fp32 -> xT packed [NQT, 128, DC, ST] bf16 st-major
        xb = np.asarray(x, np.float32).astype(bf)
        return np.ascontiguousarray(
            xb.T.reshape(DC, P, NQT, ST).transpose(2, 1, 0, 3)
        )

    def packX8(x):  # [S, D] fp32 -> [NQT, 128, DCP, 2, ST] e4m3 st-major
        xb = np.asarray(x, np.float32).T.astype(e4)  # [D, S]
        return np.ascontiguousarray(
            xb.reshape(DCP, 2, P, NQT, ST).transpose(3, 2, 0, 1, 4)
        )

    def packW8(w):  # [D, NHL] -> [128, DCP, 2, NHL] e4m3, pre-scaled x64
        wb = (np.asarray(w, np.float32) * WS).astype(e4)
        return np.ascontiguousarray(wb.reshape(DCP, 2, P, NHL).transpose(2, 0, 1, 3))

    def packW(w):  # [D, NHL] -> [128, DC, NHL]
        wb = np.asarray(w, np.float32).astype(bf)
        return np.ascontiguousarray(wb.reshape(DC, P, NHL).transpose(1, 0, 2))

    def packWo(w):  # [NHL, D] -> [128, MC, D]
        wb = np.asarray(w, np.float32).astype(bf)
        return np.ascontiguousarray(wb.reshape(MC, P, D).transpose(1, 0, 2))

    query = np.asarray(query, np.float32)
    key = np.asarray(key, np.float32)
    value = np.asarray(value, np.float32)
    in_maps = []
    for c in range(NCORES):
        b, hg = divmod(c, 2)
        cols = slice(hg * NHL, (hg + 1) * NHL)
        in_maps.append(
            {
                "xq8": packX8(query[b]),
                "xk8": packX8(key[b]),
                "xvT": packT(value[b]),
                "wq8": packW8(np.asarray(w_q)[:, cols]),
                "wk8": packW8(np.asarray(w_k)[:, cols]),
                "wv": packW(np.asarray(w_v)[:, cols]),
                "wo": packWo(np.asarray(w_o)[cols, :]),
            }
        )
    return in_maps


def kernel(query, key, value, mask, w_q, w_k, w_v, w_o, **run_kwargs):
    nc = build_nc()
    in_maps = make_in_maps(query, key, value, w_q, w_k, w_v, w_o)
    res = run_bass_kernel_spmd(nc, in_maps, list(range(NCORES)), **run_kwargs)
    out = np.empty((B, S, D), np.float32)
    for b in range(B):
        out[b] = np.asarray(res.results[2 * b]["out"], np.float32) + np.asarray(
            res.results[2 * b + 1]["out"], np.float32
        )
    return out

